# revision 14
# baseline (speedup 1.0000x reference)
"""Causal self-attention (QKV GEMM + RoPE + causal softmax attention + output
projection) for Trainium2, sharded over 8 NeuronCores.

Sharding: tensor-parallel over heads (2 heads/core). Each core computes the
QKV projections for its heads (full token range), RoPE, causal attention, and
a partial output projection over its heads' channels; the host sums the 8
partial projections (the only cross-core reduction) and reshapes.

Schedule: three software-pipelined stages keep the in-order PE queue dense:
  stage 1: QKV(b0)
  stage 2: attention(b0) interleaved with QKV(b1)   (exp hides under GEMMs)
  stage 3: attention(b1) interleaved with proj(b0) and proj(b1)
Softmax denominators avoid the PE: `at` tiles are accumulated on DVE (fp16
2x mode), partition-reduced on GpSimd, reciprocal on DVE. All inputs are
pre-converted to fp16 on the host (halves input DMA); partial outputs are
written fp16 (halves output DMA). Matmul operands fp16, accumulation fp32.
"""

import os
import sys

import numpy as np


def _ensure_concourse():
    try:
        import concourse.bass  # noqa: F401
        return
    except ImportError:
        pass
    for p in (
        "/opt/trn_rl_repo",
        os.path.expanduser("~/.axon_site/_ro/trn_rl_repo"),
        "/root/.axon_site/_ro/trn_rl_repo",
    ):
        if os.path.isdir(p) and p not in sys.path:
            sys.path.insert(0, p)
    import concourse.bass  # noqa: F401


# Problem shape (hardcoded per contract)
B, T, C, H = 2, 2048, 2048, 16
D, RD = 128, 64
NCORES = 8
HPC = H // NCORES          # heads per core = 2
BT = B * T                 # 4096
P = 128
MT = T // P                # 16 token tiles per batch
KTC = C // P               # 16 contraction tiles over C
FPC = 3 * HPC * D          # 768 qkv features per core
NQ = 512                   # query chunk
NJ = T // NQ               # 4 query chunks per instance
GRP = 2                    # m-tiles per xc DMA group
SCALE = 1.0 / float(np.sqrt(D))

_PROGRAM = None


def _build_program():
    _ensure_concourse()
    from contextlib import ExitStack

    import concourse.bacc as bacc
    import concourse.bass_isa as bass_isa
    import concourse.mybir as mybir
    import concourse.tile as tile
    from concourse.alu_op_type import AluOpType
    from concourse.masks import make_identity

    F32 = mybir.dt.float32
    F16 = mybir.dt.float16
    EXP = mybir.ActivationFunctionType.Exp
    MUL = AluOpType.mult
    SUB = AluOpType.subtract
    ADD = AluOpType.add
    RADD = bass_isa.ReduceOp.add
    PSUM = "PSUM"

    nc = bacc.Bacc("TRN2", target_bir_lowering=False, debug=False,
                   num_devices=NCORES)

    xt_d = nc.dram_tensor("xt", [C, BT], F16, kind="ExternalInput").ap()
    w_d = nc.dram_tensor("wqkv", [P, KTC * FPC], F16, kind="ExternalInput").ap()
    cos_d = nc.dram_tensor("cosw", [P, (BT // P) * RD], F16, kind="ExternalInput").ap()
    sin_d = nc.dram_tensor("sinw", [P, (BT // P) * RD], F16, kind="ExternalInput").ap()
    msk_d = nc.dram_tensor("maskd", [P, 4 * NQ], F16, kind="ExternalInput").ap()
    wp_d = nc.dram_tensor("wproj", [P, HPC * C], F16, kind="ExternalInput").ap()
    out_d = nc.dram_tensor("outp", [BT, C], F16, kind="ExternalOutput").ap()

    xt_r = xt_d.rearrange("(kt p) t -> p kt t", p=P)

    with tile.TileContext(nc) as tc, ExitStack() as gctx:
        ep = gctx.enter_context

        # ---- SBUF pools ----
        const = ep(tc.tile_pool(name="const", bufs=1))
        msk_sb = const.tile([P, 4 * NQ], F16, tag="msk")
        cos_sb = const.tile([P, (BT // P) * RD], F16, tag="cos")
        sin_sb = const.tile([P, (BT // P) * RD], F16, tag="sin")
        ident = const.tile([P, P], F16, tag="ident")

        wpool = ep(tc.tile_pool(name="wqkv", bufs=1))
        w_sb = wpool.tile([P, KTC * FPC], F16, tag="w")

        qkt_pool = ep(tc.tile_pool(name="qkt", bufs=2))
        v_pool = ep(tc.tile_pool(name="v", bufs=2))
        yt_pool = ep(tc.tile_pool(name="yt", bufs=1))
        yt_all = yt_pool.tile([P, B * HPC * T], F16, tag="yt")
        xcol = ep(tc.tile_pool(name="xcol", bufs=4))
        qkraw = ep(tc.tile_pool(name="qkraw", bufs=2))
        rotp = ep(tc.tile_pool(name="rot", bufs=2))
        tmpp = ep(tc.tile_pool(name="tmp", bufs=1))
        attnp = ep(tc.tile_pool(name="attn", bufs=4))
        accp = ep(tc.tile_pool(name="acc", bufs=2))
        sredp = ep(tc.tile_pool(name="sred", bufs=2))
        srecp = ep(tc.tile_pool(name="srec", bufs=2))
        outrow = ep(tc.tile_pool(name="orow", bufs=2))
        wppool = ep(tc.tile_pool(name="wp", bufs=1))
        wp_sb = wppool.tile([P, HPC * C], F16, tag="wp")

        # ---- PSUM pools.  pss+psy live for the whole kernel (4 banks);
        # the QKV-phase pools (4 banks) are closed after stage 2 and the
        # banks rebound to a 4-deep projection pool for stage 3. ----
        pss = ep(tc.tile_pool(name="pss", bufs=2, space=PSUM))    # 2 banks
        psy = ep(tc.tile_pool(name="psy", bufs=2, space=PSUM))    # 2 banks
        astack = ExitStack()
        psA = astack.enter_context(tc.tile_pool(name="psA", bufs=2,
                                                space=PSUM))      # 2 banks
        ps2 = astack.enter_context(tc.tile_pool(name="ps2", bufs=1,
                                                space=PSUM))      # 1 bank
        pstp = astack.enter_context(tc.tile_pool(name="pstp", bufs=1,
                                                 space=PSUM))     # 1 bank
        pools = {}

        # ---- preamble DMAs: the first x tiles go out first on the sync
        # ring; the weight chunks stream just-in-time behind them across
        # all three dispatch rings, so the first GEMM tile starts ~2us in
        # and never starves ----
        xcs0 = []
        for half in range(2):
            xc = xcol.tile([P, KTC // 2, P * GRP], F16, tag="xc",
                           name=f"xc_0_0_{half}")
            nc.sync.dma_start(
                out=xc[:],
                in_=xt_r[:, half * (KTC // 2):(half + 1) * (KTC // 2),
                         slice(0, GRP * P)])
            xcs0.append(xc)
        rings = [nc.scalar, nc.gpsimd, nc.sync]
        for kt in range(KTC):
            rings[kt % 3].dma_start(out=w_sb[:, kt * FPC:(kt + 1) * FPC],
                                    in_=w_d[:, kt * FPC:(kt + 1) * FPC])
        nc.gpsimd.dma_start(out=cos_sb[:], in_=cos_d)
        nc.gpsimd.dma_start(out=sin_sb[:], in_=sin_d)
        nc.scalar.dma_start(out=msk_sb[:], in_=msk_d)
        nc.scalar.dma_start(out=wp_sb[:], in_=wp_d)
        make_identity(nc, ident[:])

        qkT = [qkt_pool.tile([P, MT, 4, P], F16, tag="qkT", name=f"qkT{b}")
               for b in range(B)]
        v_sb = [v_pool.tile([P, MT, HPC, D], F16, tag="v", name=f"v{b}")
                for b in range(B)]

        def genA(b):
            """QKV GEMM + RoPE + transpose of q,k for batch b.

            Yields after each PE work unit.  Transposes are emitted one
            m-tile late (pending) so the PE never waits on RoPE."""
            pending = None  # (m, rot tile) awaiting transpose emission

            def emit_transpose(m, rot_t):
                tp = pstp.tile([P, 512], F16, tag="tp", name=f"tp_{b}_{m}")
                for hb in range(4):
                    nc.tensor.transpose(
                        tp[:, hb * P:(hb + 1) * P],
                        rot_t[:, hb * P:(hb + 1) * P], ident[:])
                nc.scalar.copy(qkT[b][:, m, :, :], tp[:])

            for g0 in range(0, MT, GRP):
                if b == 0 and g0 == 0:
                    xcs = xcs0
                else:
                    tsl = slice(b * T + g0 * P, b * T + (g0 + GRP) * P)
                    xcs = []
                    for half in range(2):
                        xc = xcol.tile([P, KTC // 2, P * GRP], F16, tag="xc",
                                       name=f"xc_{b}_{g0}_{half}")
                        nc.sync.dma_start(
                            out=xc[:],
                            in_=xt_r[:, half * (KTC // 2):
                                     (half + 1) * (KTC // 2), tsl])
                        xcs.append(xc)
                for mi in range(GRP):
                    m = g0 + mi
                    p5 = psA.tile([P, 512], F32, tag="p5",
                                  name=f"p5_{b}_{m}")
                    p2 = ps2.tile([P, 256], F32, tag="p2",
                                  name=f"p2_{b}_{m}")
                    for half in range(2):
                        for k8 in range(KTC // 2):
                            kt = half * (KTC // 2) + k8
                            lhsT = xcs[half][:, k8, mi * P:(mi + 1) * P]
                            nc.tensor.matmul(
                                p5[:], lhsT,
                                w_sb[:, kt * FPC: kt * FPC + 512],
                                start=(kt == 0), stop=(kt == KTC - 1))
                            nc.tensor.matmul(
                                p2[:], lhsT,
                                w_sb[:, kt * FPC + 512:(kt + 1) * FPC],
                                start=(kt == 0), stop=(kt == KTC - 1))
                            yield
                    # evict + RoPE for this m-tile right away; its
                    # transposes go out one m-tile later
                    gm = b * MT + m
                    qkr = qkraw.tile([P, 512], F16, tag="qkr",
                                     name=f"qkr_{b}_{m}")
                    nc.scalar.copy(qkr[:], p5[:])
                    nc.scalar.copy(v_sb[b][:, m, :, :], p2[:])
                    rot_t = rotp.tile([P, 512], F16, tag="rot",
                                      name=f"rot_{b}_{m}")
                    q3 = qkr[:].rearrange(
                        "p (blk two d) -> p blk two d", two=2, d=RD)
                    re_, im_ = q3[:, :, 0, :], q3[:, :, 1, :]
                    r3 = rot_t[:].rearrange(
                        "p (blk two d) -> p blk two d", two=2, d=RD)
                    cosb = (cos_sb[:, gm * RD:(gm + 1) * RD]
                            .unsqueeze(1).broadcast_to([P, 4, RD]))
                    sinb = (sin_sb[:, gm * RD:(gm + 1) * RD]
                            .unsqueeze(1).broadcast_to([P, 4, RD]))
                    t1 = tmpp.tile([P, 256], F16, tag="t1",
                                   name=f"t1_{b}_{m}")
                    t2 = tmpp.tile([P, 256], F16, tag="t2",
                                   name=f"t2_{b}_{m}")
                    t1v = t1[:].rearrange("p (blk d) -> p blk d", d=RD)
                    t2v = t2[:].rearrange("p (blk d) -> p blk d", d=RD)
                    nc.vector.tensor_tensor(t1v, re_, cosb, MUL)
                    nc.vector.tensor_tensor(t2v, im_, sinb, MUL)
                    nc.vector.tensor_tensor(r3[:, :, 0, :], t1v, t2v, SUB)
                    t3 = tmpp.tile([P, 256], F16, tag="t3",
                                   name=f"t3_{b}_{m}")
                    t4 = tmpp.tile([P, 256], F16, tag="t4",
                                   name=f"t4_{b}_{m}")
                    t3v = t3[:].rearrange("p (blk d) -> p blk d", d=RD)
                    t4v = t4[:].rearrange("p (blk d) -> p blk d", d=RD)
                    nc.vector.tensor_tensor(t3v, re_, sinb, MUL)
                    nc.vector.tensor_tensor(t4v, im_, cosb, MUL)
                    nc.vector.tensor_tensor(r3[:, :, 1, :], t3v, t4v, ADD)
                    if pending is not None:
                        emit_transpose(*pending)
                    pending = (m, rot_t)
                    yield
            if pending is not None:
                emit_transpose(*pending)

        def finalize(inst, j, y_ps, acc):
            # softmax denominator: partition-reduce the accumulated fp16
            # attention weights on GpSimd (result broadcast to all
            # partitions), reciprocal + scale on DVE.  No PE involvement.
            sred = sredp.tile([P, NQ], F32, tag="sr", name=f"sr_{inst}_{j}")
            nc.gpsimd.partition_all_reduce(sred[:], acc[:], channels=P,
                                           reduce_op=RADD)
            srec = srecp.tile([P, NQ], F32, tag="sc2", name=f"sc2_{inst}_{j}")
            with nc.allow_low_precision(reason="softmax recip"):
                nc.vector.reciprocal_approx_fast(out=srec[:], in_=sred[:])
            nc.vector.tensor_tensor(
                yt_all[:, inst * T + j * NQ: inst * T + (j + 1) * NQ],
                y_ps[:], srec[:], MUL)

        def genB(b, prog):
            """Causal attention for batch b; j outer / head inner so the
            proj of earlier token tiles unblocks early.  y-matmuls are
            emitted 2 kt-steps late so exp latency never stalls the PE."""
            pending = None
            for j in range(NJ):
                for h in range(HPC):
                    inst = b * HPC + h
                    y_ps = psy.tile([P, NQ], F32, tag="y",
                                    name=f"y_{inst}_{j}")
                    acc = accp.tile([P, NQ], F16, tag="acc",
                                    name=f"acc_{inst}_{j}")
                    nkt = 4 * (j + 1)
                    lag = []

                    def emit_y(kt, at):
                        nc.tensor.matmul(
                            y_ps[:], v_sb[b][:, kt, h, :], at[:],
                            start=(kt == 0), stop=(kt == nkt - 1))

                    for kt in range(nkt):
                        sc = pss.tile([P, NQ], F32, tag="sc",
                                      name=f"sc_{inst}_{j}_{kt}")
                        nc.tensor.matmul(
                            sc[:], qkT[b][:, kt, 2 + h, :],
                            qkT[b][:, 4 * j:4 * (j + 1), h, :],
                            start=True, stop=True)
                        at = attnp.tile([P, NQ], F16, tag="at",
                                        name=f"at_{inst}_{j}_{kt}")
                        nc.scalar.activation(at[:], sc[:], EXP, scale=SCALE)
                        if kt >= nkt - 4:
                            i = kt - (nkt - 4)
                            nc.vector.tensor_tensor(
                                at[:], at[:],
                                msk_sb[:, i * NQ:(i + 1) * NQ], MUL)
                        if kt == 0:
                            nc.vector.tensor_copy(acc[:], at[:])
                        else:
                            nc.vector.tensor_tensor(acc[:], acc[:], at[:],
                                                    ADD)
                        lag.append((kt, at))
                        if len(lag) > 2:
                            emit_y(*lag.pop(0))
                        if kt == 1 and pending is not None:
                            finalize(*pending)
                            if pending[0] == b * HPC + 1:  # h==1 done
                                prog[b] = pending[1]
                            pending = None
                        yield
                    for e in lag:
                        emit_y(*e)
                    pending = (inst, j, y_ps, acc)
            if pending is not None:
                finalize(*pending)
                if pending[0] == b * HPC + 1:
                    prog[b] = pending[1]

        def genC(b, prog):
            """Partial output projection for batch b; stalls until the
            attention outputs it reads have been finalized."""
            for m in range(MT):
                while prog[b] < m // 4:
                    yield "stall"
                orow = outrow.tile([P, C], F16, tag="orow",
                                   name=f"orow_{b}_{m}")
                for oc in range(4):
                    op = pools["pso"].tile([P, 512], F32, tag="op",
                                           name=f"op_{b}_{m}_{oc}")
                    for h in range(HPC):
                        nc.tensor.matmul(
                            op[:],
                            yt_all[:, (b * HPC + h) * T + m * P:
                                   (b * HPC + h) * T + (m + 1) * P],
                            wp_sb[:, h * C + oc * 512: h * C + (oc + 1) * 512],
                            start=(h == 0), stop=(h == HPC - 1))
                    if oc % 2 == 0:
                        nc.scalar.copy(orow[:, oc * 512:(oc + 1) * 512],
                                       op[:])
                    else:
                        nc.vector.tensor_copy(orow[:, oc * 512:(oc + 1) * 512],
                                              op[:])
                    yield
                nc.sync.dma_start(
                    out=out_d[(b * MT + m) * P:(b * MT + m + 1) * P, :],
                    in_=orow[:])

        def interleave(main, fill, ratio):
            debt = 0.0
            done = False
            for _ in main:
                if done:
                    continue
                debt += ratio
                while debt >= 1.0:
                    step = next(fill, "end")
                    if step == "end":
                        done = True
                        break
                    if step == "stall":
                        break
                    debt -= 1.0
            if not done:
                for _ in fill:
                    pass

        def chain(*gens):
            for g in gens:
                yield from g

        def prefer(g1, g2):
            """Yield from g1 while it's ready; take g2 steps while g1 is
            stalled; finish with whatever remains."""
            while True:
                s = next(g1, "end")
                if s == "end":
                    yield from g2
                    return
                if s == "stall":
                    s2 = next(g2, "end")
                    yield ("stall" if s2 == "end" else s2)
                else:
                    yield s

        prog = {0: -1, 1: -1}
        # stage 1: QKV(b0)
        for _ in genA(0):
            pass
        # stage 2: attention(b0) with QKV(b1) filling PE gaps
        interleave(genB(0, prog), genA(1), ratio=272.0 / 80.0)
        # stage 3: attention(b1) with proj(b0) then proj(b1) filling
        astack.close()
        pools["pso"] = gctx.enter_context(
            tc.tile_pool(name="pso", bufs=4, space=PSUM))         # 4 banks
        interleave(genB(1, prog), prefer(genC(1, prog), genC(0, prog)),
                   ratio=128.0 / 80.0)

    nc.compile()
    return nc


def _perm(rows):
    return np.concatenate([rows[0::2], rows[1::2]], axis=0)


def _host_inputs(x, mask, freqs_cos, freqs_sin, w_attn, w_proj):
    f16 = np.float16
    f32 = np.float32
    x = np.asarray(x, f32)
    mask = np.asarray(mask)
    fc = np.asarray(freqs_cos, f32)
    fs = np.asarray(freqs_sin, f32)
    w_attn = np.asarray(w_attn, f32)
    w_proj = np.asarray(w_proj, f32)

    xT = np.ascontiguousarray(x.reshape(BT, C).T.astype(f16))

    def rows_arrange(a):  # [BT, RD] -> [P, (BT//P)*RD]
        return np.ascontiguousarray(
            a.reshape(BT // P, P, RD).transpose(1, 0, 2).reshape(P, -1)
            .astype(f16))

    cosw = rows_arrange(np.concatenate([fc] * B, axis=0))
    sinw = rows_arrange(np.concatenate([fs] * B, axis=0))

    maskd = np.concatenate(
        [mask[0:NQ, i * P:(i + 1) * P].T.astype(f16) for i in range(4)],
        axis=1)
    maskd = np.ascontiguousarray(maskd)

    wq, wk, wv = w_attn[0:C], w_attn[C:2 * C], w_attn[2 * C:3 * C]
    in_maps = []
    for c in range(NCORES):
        h0, h1 = HPC * c, HPC * c + 1
        Wc = np.concatenate([
            _perm(wq[h0 * D:(h0 + 1) * D]), _perm(wq[h1 * D:(h1 + 1) * D]),
            _perm(wk[h0 * D:(h0 + 1) * D]), _perm(wk[h1 * D:(h1 + 1) * D]),
            wv[h0 * D:(h0 + 1) * D], wv[h1 * D:(h1 + 1) * D]], axis=0)
        wqkv_c = np.ascontiguousarray(
            Wc.T.reshape(KTC, P, FPC).transpose(1, 0, 2).reshape(P, KTC * FPC)
            .astype(f16))
        wp_c = w_proj[:, c * HPC * D:(c + 1) * HPC * D].T  # [256, C]
        wp_c = np.ascontiguousarray(
            wp_c.reshape(HPC, P, C).transpose(1, 0, 2).reshape(P, HPC * C)
            .astype(f16))
        in_maps.append({
            "xt": xT, "wqkv": wqkv_c, "cosw": cosw, "sinw": sinw,
            "maskd": maskd, "wproj": wp_c,
        })
    return in_maps


def kernel(x, mask, freqs_cos, freqs_sin, w_attn, w_proj):
    global _PROGRAM
    _ensure_concourse()
    from concourse.bass_utils import run_bass_kernel_spmd

    if _PROGRAM is None:
        _PROGRAM = _build_program()
    nc = _PROGRAM

    in_maps = _host_inputs(x, mask, freqs_cos, freqs_sin, w_attn, w_proj)
    res = run_bass_kernel_spmd(nc, in_maps, list(range(NCORES)))
    out = res.results[0]["outp"].astype(np.float64)
    for i in range(1, NCORES):
        out = out + res.results[i]["outp"]
    return np.ascontiguousarray(out.reshape(B, T, C).astype(np.float32))
